# revision 2
# baseline (speedup 1.0000x reference)
"""Trainium2 Bass kernel: BN(eval) -> sign -> Conv1d(K=7,pad=3) -> alpha -> PReLU -> MaxPool2.

Strategy (hardcoded for B=64, CIN=64, L=4096, COUT=128, K=7):
  - Data-parallel over batch: 8 samples per NeuronCore x 8 cores; no
    cross-core communication.
  - Host folds BN into per-channel (scale, bias) and alpha into the conv
    weights (bf16); PReLU slope rides in as an SBUF vector.
  - A PAIR of samples shares the 128 partitions: rows 0-63 = sample A,
    rows 64-127 = sample B. Sign tiles are PER 512-col OUTPUT TILE
    (518 cols incl. the K-1=6 halo), so the first matmuls only wait for
    chunk 0's DMA+sign instead of a whole pair.
  - Conv = 7 PSUM-accumulated K=64 bf16 matmuls per 512-col tile; sample
    A on PE row-group 0-1, B on 2-3 concurrently (weights duplicated in
    both halves) -> ~100% of bf16 peak in steady state (216ns/slot).
  - HAM warm-up: ~40 tiny matmuls on a memset-only tile start at t=0
    (no DMA dependency), flipping the PE clock gate to 8/8 before the
    first real matmul arrives.
  - MaxPool(2) straight out of PSUM via DVE tensor_reduce(max); PReLU
    applied AFTER pooling (they commute) via ScalarE Prelu on coarse
    spans; the final pair flushes in fine pieces with the last two
    prelus split Scalar/Vector to shorten the tail.
  - DMA queues: input chunks on Sync, outputs on GpSimd (idle engine),
    consts on Scalar/Sync first so nothing gates on them late.
  - Walrus in this toolchain accepts only one sync-wait per instruction,
    so the Tile-scheduled BIR is post-processed (_split_sync_waits_json).
"""

import json
import sys

for _p in ("/opt/trn_rl_repo", "/root/.axon_site/_ro/trn_rl_repo"):
    if _p not in sys.path:
        sys.path.append(_p)

import numpy as np
import ml_dtypes

import concourse.bass as bass
import concourse.tile as tile
from concourse import mybir
from concourse.bass_utils import run_bass_kernel_spmd

B, CIN, L, COUT, K = 64, 64, 4096, 128, 7
PAD = 3
BN_EPS = 1e-5
N_CORES = 8
BPC = B // N_CORES  # samples per core
LOUT = L // 2       # 2048 pooled length
NT = L // 512       # 8 output tiles of 512 cols
SGW = 512 + K - 1   # per-tile sign width incl. halo = 518

_CACHE: dict = {}


def build_program() -> "bass.Bass":
    nc = bass.Bass(trn_type="TRN2")
    I8 = nc.dram_tensor("I8", [BPC, CIN, L], mybir.dt.float32, kind="ExternalInput")
    W = nc.dram_tensor("W", [128, K * 128], mybir.dt.bfloat16, kind="ExternalInput")
    SBp = nc.dram_tensor("SBp", [128, 4], mybir.dt.float32, kind="ExternalInput")
    O8 = nc.dram_tensor("O8", [BPC, COUT, LOUT], mybir.dt.bfloat16, kind="ExternalOutput")

    iflat = I8.ap().flatten_outer_dims()  # [BPC*64, 4096]
    oflat = O8.ap().flatten_outer_dims()  # [BPC*128, 2048]

    AF = mybir.ActivationFunctionType
    with tile.TileContext(nc) as tc:
        with (
            tc.tile_pool(name="consts", bufs=1) as consts,
            tc.tile_pool(name="ipair", bufs=6) as ipool,
            tc.tile_pool(name="sgn", bufs=16) as spool,
            tc.tile_pool(name="pooled", bufs=2) as plpool,
            tc.tile_pool(name="outp", bufs=4) as opool,
            tc.tile_pool(name="ps", bufs=8, space="PSUM") as pspool,
        ):
            # --- consts + warmup (no DMA dependencies for the warmup) ---
            warm_lhs = consts.tile([128, 128], mybir.dt.bfloat16)
            nc.gpsimd.memset(warm_lhs[:], 0.0)
            # dummy activation hoists the ACT table load to kernel start
            dummy = consts.tile([128, 4], mybir.dt.float32)
            nc.scalar.activation(dummy[:], warm_lhs[:, 0:4], AF.Sign)

            sb_sb = consts.tile([128, 4], mybir.dt.float32)
            nc.sync.dma_start(sb_sb[:], SBp.ap()[:])
            w_sb = consts.tile([128, K * 128], mybir.dt.bfloat16)
            nc.scalar.dma_start(w_sb[:], W.ap()[:])

            # HAM warmup: ~40 64-col matmuls (~4.3us cold) flip the PE
            # clock gate to 8/8 before the first real matmul is ready
            warm = pspool.tile([128, 512], mybir.dt.float32, name="warm", tag="psb")
            for _ in range(40):
                nc.tensor.matmul(
                    warm[:, 0:64], warm_lhs[0:64, :], warm_lhs[0:64, 0:64],
                    start=True, stop=True,
                )

            sgn_scale = sb_sb[:, 0:1]
            sgn_bias = sb_sb[:, 1:2]
            slope = sb_sb[:, 3:4]

            for t in range(BPC // 2):
                last_pair = t == BPC // 2 - 1
                # --- per-tile input DMA + sign (chunk c covers sign-space
                # cols [512c, 512c+517] <- input cols [512c-3, 512c+514]) ---
                sgc = []
                for c in range(NT):
                    sg = spool.tile([128, SGW], mybir.dt.bfloat16, name=f"sg{c}", tag=f"sg{c}")
                    x0 = 512 * c - PAD
                    d0 = 0
                    w = SGW
                    if c == 0:
                        nc.gpsimd.memset(sg[:, 0:PAD], 0.0)
                        x0, d0, w = 0, PAD, SGW - PAD
                    elif c == NT - 1:
                        nc.gpsimd.memset(sg[:, SGW - PAD : SGW], 0.0)
                        w = SGW - PAD
                    ipc = ipool.tile([128, SGW], mybir.dt.float32, name="ipc", tag="ipc")
                    nc.sync.dma_start(
                        ipc[:, 0:w], iflat[128 * t : 128 * (t + 1), x0 : x0 + w]
                    )
                    nc.scalar.activation(
                        sg[:, d0 : d0 + w], ipc[:, 0:w],
                        AF.Sign, bias=sgn_bias, scale=sgn_scale,
                    )
                    sgc.append(sg)

                pla = plpool.tile([128, LOUT], mybir.dt.bfloat16, name="pla", tag="pla")
                plb = plpool.tile([128, LOUT], mybir.dt.bfloat16, name="plb", tag="plb")
                for it in range(NT):
                    psa = pspool.tile([128, 512], mybir.dt.float32, name="psa", tag="psb")
                    psb = pspool.tile([128, 512], mybir.dt.float32, name="psb", tag="psb")
                    sg = sgc[it]
                    for k in range(K):
                        nc.tensor.matmul(
                            psa[:], w_sb[0:64, 128 * k : 128 * (k + 1)],
                            sg[0:64, k : k + 512],
                            start=(k == 0), stop=(k == K - 1),
                        )
                        nc.tensor.matmul(
                            psb[:], w_sb[64:128, 128 * k : 128 * (k + 1)],
                            sg[64:128, k : k + 512],
                            start=(k == 0), stop=(k == K - 1),
                        )
                    o0 = 256 * it
                    nc.vector.tensor_reduce(
                        pla[:, o0 : o0 + 256],
                        psa[:].rearrange("p (n two) -> p n two", two=2),
                        mybir.AxisListType.X,
                        mybir.AluOpType.max,
                    )
                    nc.vector.tensor_reduce(
                        plb[:, o0 : o0 + 256],
                        psb[:].rearrange("p (n two) -> p n two", two=2),
                        mybir.AxisListType.X,
                        mybir.AluOpType.max,
                    )
                    # pooled-span flush: prelu (commutes with max) + out DMA.
                    # Non-last pairs: coarse [1024] spans. Last pair: finer
                    # pieces, final ones split Scalar/Vector to cut the tail.
                    if not last_pair:
                        spans = []
                        if it == 3:
                            spans = [(0, 1024, "s", "g")]
                        elif it == 7:
                            spans = [(1024, 1024, "s", "g")]
                        for s0, sw, pe, qe in spans:
                            for pl, b in ((pla, 2 * t), (plb, 2 * t + 1)):
                                o = opool.tile([128, sw], mybir.dt.bfloat16, name="o", tag="o")
                                nc.scalar.activation(
                                    o[:], pl[:, s0 : s0 + sw], AF.Prelu, alpha=slope
                                )
                                nc.gpsimd.dma_start(
                                    oflat[128 * b : 128 * (b + 1), s0 : s0 + sw], o[:]
                                )
                    else:
                        spans = []
                        if it == 3:
                            spans = [(0, 1024)]
                        elif it == 5:
                            spans = [(1024, 512)]
                        elif it == 6:
                            spans = [(1536, 256)]
                        elif it == 7:
                            spans = [(1792, 256)]
                        for s0, sw in spans:
                            final = it == 7
                            # sample A: prelu on Scalar, DMA on GpSimd
                            oa = opool.tile([128, sw], mybir.dt.bfloat16, name="oa", tag="o")
                            nc.scalar.activation(
                                oa[:], pla[:, s0 : s0 + sw], AF.Prelu, alpha=slope
                            )
                            nc.gpsimd.dma_start(
                                oflat[128 * 2 * t : 128 * (2 * t + 1), s0 : s0 + sw],
                                oa[:],
                            )
                            # sample B: final piece -> prelu on Vector (stt),
                            # DMA on Sync; earlier pieces same as A
                            ob = opool.tile([128, sw], mybir.dt.bfloat16, name="ob", tag="o")
                            if final:
                                nc.vector.scalar_tensor_tensor(
                                    ob[:], plb[:, s0 : s0 + sw], slope,
                                    plb[:, s0 : s0 + sw],
                                    mybir.AluOpType.mult, mybir.AluOpType.max,
                                )
                                nc.sync.dma_start(
                                    oflat[128 * (2 * t + 1) : 128 * (2 * t + 2), s0 : s0 + sw],
                                    ob[:],
                                )
                            else:
                                nc.scalar.activation(
                                    ob[:], plb[:, s0 : s0 + sw], AF.Prelu, alpha=slope
                                )
                                nc.gpsimd.dma_start(
                                    oflat[128 * (2 * t + 1) : 128 * (2 * t + 2), s0 : s0 + sw],
                                    ob[:],
                                )
    return nc


def _split_sync_waits_json(bir: bytes) -> bytes:
    """Walrus in this toolchain accepts at most one sync-wait per instruction.
    Hoist multi-wait sync_info lists into preceding single-wait EventSemaphore
    instructions on the same engine queue (the same form engine.wait_ge()
    lowers to), preserving program order and on_update placement."""
    j = json.loads(bir)
    n_split = 0
    for fn in j.get("functions", []):
        for blk in fn.get("blocks", []):
            ins_list = blk.get("instructions")
            if not ins_list:
                continue
            out = []
            for ins in ins_list:
                si = ins.get("sync_info")
                waits = si.get("on_wait") if si else None
                if waits and len(waits) > 1:
                    for i, w in enumerate(waits):
                        out.append(
                            {
                                "debug": ins.get("debug", 0),
                                "engine": ins["engine"],
                                "ins": [],
                                "outs": [],
                                "name": f"{ins['name']}-antw{i}",
                                "opcode": "EventSemaphore",
                                "sync_info": {"on_update": [], "on_wait": [w]},
                            }
                        )
                    si["on_wait"] = []
                    n_split += 1
                out.append(ins)
            blk["instructions"] = out
    return json.dumps(j).encode()


def get_program() -> "bass.Bass":
    if "nc" not in _CACHE:
        nc = build_program()
        orig = nc.to_json_bytes
        nc.to_json_bytes = lambda: _split_sync_waits_json(orig())
        _CACHE["nc"] = nc
    return _CACHE["nc"]


def prep_inputs(I, bn_gamma, bn_beta, bn_mean, bn_var, conv_w, alpha, prelu_w):
    """Host-side folding: BN -> (scale, bias); alpha -> weights; per-k lhsT
    blocks duplicated into both PE array halves."""
    f32 = np.float32
    gamma = np.asarray(bn_gamma, f32)
    beta = np.asarray(bn_beta, f32)
    mean = np.asarray(bn_mean, f32)
    var = np.asarray(bn_var, f32)
    s = gamma / np.sqrt(var + f32(BN_EPS))        # [CIN]
    t = beta - mean * s                            # [CIN]

    w = np.asarray(conv_w, f32) * np.asarray(alpha, f32)[:, None, None]  # [COUT, CIN, K]
    Wb = np.zeros((128, K * 128), np.float32)
    for k in range(K):
        Wb[0:64, 128 * k : 128 * k + 128] = w[:, :, k].T
        Wb[64:128, 128 * k : 128 * k + 128] = w[:, :, k].T
    Wb = Wb.astype(ml_dtypes.bfloat16)

    a = f32(np.asarray(prelu_w, f32).reshape(-1)[0])
    sbp = np.zeros((128, 4), f32)
    sbp[0:64, 0] = s
    sbp[64:128, 0] = s
    sbp[0:64, 1] = t
    sbp[64:128, 1] = t
    sbp[:, 2] = f32(1.0) - a
    sbp[:, 3] = a
    return Wb, sbp


def kernel(I, bn_gamma, bn_beta, bn_mean, bn_var, conv_w, alpha, prelu_w):
    I = np.ascontiguousarray(np.asarray(I, np.float32))
    assert I.shape == (B, CIN, L), I.shape
    Wb, sbp = prep_inputs(I, bn_gamma, bn_beta, bn_mean, bn_var, conv_w, alpha, prelu_w)

    nc = get_program()
    in_maps = [
        {"I8": I[BPC * c : BPC * (c + 1)], "W": Wb, "SBp": sbp} for c in range(N_CORES)
    ]
    res = run_bass_kernel_spmd(nc, in_maps, core_ids=list(range(N_CORES)))
    out = np.concatenate(
        [np.asarray(res.results[c]["O8"]) for c in range(N_CORES)], axis=0
    )
    return np.ascontiguousarray(out.astype(np.float32))


# revision 7
# speedup vs baseline: 1.0849x; 1.0849x over previous
"""Trainium2 Bass kernel: BN(eval) -> sign -> Conv1d(K=7,pad=3) -> alpha -> PReLU -> MaxPool2.

Strategy (hardcoded for B=64, CIN=64, L=4096, COUT=128, K=7):
  - Data-parallel over batch: 8 samples per NeuronCore x 8 cores; no
    cross-core communication.
  - Host folds BN into per-channel (scale, bias) and alpha into the conv
    weights (bf16); PReLU slope rides in as an SBUF vector.
  - A PAIR of samples shares the 128 partitions: rows 0-63 = sample A,
    rows 64-127 = sample B. Sign tiles are PER 512-col OUTPUT TILE
    (518 cols incl. the K-1=6 halo), so the first matmuls only wait for
    chunk 0's DMA+sign instead of a whole pair.
  - Conv = 7 PSUM-accumulated K=64 bf16 matmuls per 512-col tile; sample
    A on PE row-group 0-1, B on 2-3 concurrently (weights duplicated in
    both halves) -> ~100% of bf16 peak in steady state (216ns/slot).
  - HAM warm-up: ~40 tiny matmuls on a memset-only tile start at t=0
    (no DMA dependency), flipping the PE clock gate to 8/8 before the
    first real matmul arrives.
  - MaxPool(2) straight out of PSUM via DVE tensor_reduce(max); PReLU
    applied AFTER pooling (they commute) via ScalarE Prelu on coarse
    spans; the final pair flushes in fine pieces with the last two
    prelus split Scalar/Vector to shorten the tail.
  - DMA queues: input chunks on Sync, outputs on GpSimd (idle engine),
    consts on Scalar/Sync first so nothing gates on them late.
  - Walrus in this toolchain accepts only one sync-wait per instruction,
    so the Tile-scheduled BIR is post-processed (_split_sync_waits_json).
"""

import json
import sys

for _p in ("/opt/trn_rl_repo", "/root/.axon_site/_ro/trn_rl_repo"):
    if _p not in sys.path:
        sys.path.append(_p)

import numpy as np
import ml_dtypes

import concourse.bass as bass
import concourse.tile as tile
from concourse import mybir
from concourse.bass_utils import run_bass_kernel_spmd

B, CIN, L, COUT, K = 64, 64, 4096, 128, 7
PAD = 3
BN_EPS = 1e-5
N_CORES = 8
BPC = B // N_CORES  # samples per core
LOUT = L // 2       # 2048 pooled length
NT = L // 512       # 8 output tiles of 512 cols
SGW = 512 + K - 1   # per-tile sign width incl. halo = 518

_CACHE: dict = {}


def build_program() -> "bass.Bass":
    nc = bass.Bass(trn_type="TRN2")
    I8 = nc.dram_tensor("I8", [BPC, CIN, L], mybir.dt.float32, kind="ExternalInput")
    W = nc.dram_tensor("W", [128, K * 128], mybir.dt.bfloat16, kind="ExternalInput")
    SBp = nc.dram_tensor("SBp", [128, 4], mybir.dt.float32, kind="ExternalInput")
    O8 = nc.dram_tensor("O8", [BPC, COUT, LOUT], mybir.dt.bfloat16, kind="ExternalOutput")

    iflat = I8.ap().flatten_outer_dims()  # [BPC*64, 4096]
    oflat = O8.ap().flatten_outer_dims()  # [BPC*128, 2048]

    AF = mybir.ActivationFunctionType
    with tile.TileContext(nc) as tc:
        with (
            tc.tile_pool(name="consts", bufs=1) as consts,
            tc.tile_pool(name="ipair", bufs=10) as ipool,
            tc.tile_pool(name="sgn", bufs=3) as spool,
            tc.tile_pool(name="pooled", bufs=3) as plpool,
            tc.tile_pool(name="outp", bufs=4) as opool,
            tc.tile_pool(name="ps", bufs=8, space="PSUM") as pspool,
        ):
            # --- consts + warmup (no DMA dependencies for the warmup) ---
            warm_lhs = consts.tile([128, 128], mybir.dt.bfloat16)
            nc.gpsimd.memset(warm_lhs[:], 0.0)
            # dummy activation hoists the ACT table load to kernel start
            dummy = consts.tile([128, 4], mybir.dt.float32)
            nc.scalar.activation(dummy[:], warm_lhs[:, 0:4], AF.Sign)

            sb_sb = consts.tile([128, 4], mybir.dt.float32)
            nc.sync.dma_start(sb_sb[:], SBp.ap()[:])
            w_sb = consts.tile([128, K * 128], mybir.dt.bfloat16)
            nc.scalar.dma_start(w_sb[:], W.ap()[:])

            # HAM warmup: ~88 64-col matmuls (~56ns each) span the PE boot
            # -> first-real-matmul window (~7.7us -> ~13us), flipping the
            # clock gate to 8/8 before real work and never letting the PE
            # idle long enough to re-throttle
            warm = pspool.tile([128, 512], mybir.dt.float32, name="warm", tag="psb")
            for _ in range(88):
                nc.tensor.matmul(
                    warm[:, 0:64], warm_lhs[0:64, :], warm_lhs[0:64, 0:64],
                    start=True, stop=True,
                )

            sgn_scale = sb_sb[:, 0:1]
            sgn_bias = sb_sb[:, 1:2]
            slope = sb_sb[:, 3:4]

            for t in range(BPC // 2):
                last_pair = t == BPC // 2 - 1
                # --- chunked input DMA + sign. Chunk covering output tiles
                # [t0, t0+nt) spans sign-space cols [512*t0, 512*(t0+nt)+5]
                # <- input cols [512*t0-3, ...]. Pair 0 uses fine 1-tile
                # chunks (fast first matmul); later pairs use coarse 4-tile
                # chunks (fewer, cheaper ScalarE activations). ---
                chunk_nt = 1 if t == 0 else 4
                sgc = []  # tile -> (sg tile, col offset of tile in chunk)
                for c0 in range(0, NT, chunk_nt):
                    cw = 512 * chunk_nt + K - 1
                    sg = spool.tile(
                        [128, cw], mybir.dt.bfloat16,
                        name=f"sg{chunk_nt}_{c0}", tag=f"sg{chunk_nt}_{c0}",
                    )
                    x0 = 512 * c0 - PAD
                    d0, w = 0, cw
                    if c0 == 0:
                        nc.gpsimd.memset(sg[:, 0:PAD], 0.0)
                        x0, d0, w = 0, PAD, cw - PAD
                    if c0 + chunk_nt == NT:
                        nc.gpsimd.memset(sg[:, cw - PAD : cw], 0.0)
                        w -= PAD
                    ipc = ipool.tile(
                        [128, cw], mybir.dt.float32, name=f"ipc{chunk_nt}", tag=f"ipc{chunk_nt}"
                    )
                    nc.sync.dma_start(
                        ipc[:, 0:w], iflat[128 * t : 128 * (t + 1), x0 : x0 + w]
                    )
                    nc.scalar.activation(
                        sg[:, d0 : d0 + w], ipc[:, 0:w],
                        AF.Sign, bias=sgn_bias, scale=sgn_scale,
                    )
                    for j in range(chunk_nt):
                        sgc.append((sg, 512 * j))

                pla = plpool.tile([128, LOUT], mybir.dt.bfloat16, name="pla", tag="pla")
                plb = plpool.tile([128, LOUT], mybir.dt.bfloat16, name="plb", tag="plb")
                for it in range(NT):
                    psa = pspool.tile([128, 512], mybir.dt.float32, name="psa", tag="psb")
                    psb = pspool.tile([128, 512], mybir.dt.float32, name="psb", tag="psb")
                    sg, off = sgc[it]
                    for k in range(K):
                        c0 = off + k
                        nc.tensor.matmul(
                            psa[:], w_sb[0:64, 128 * k : 128 * (k + 1)],
                            sg[0:64, c0 : c0 + 512],
                            start=(k == 0), stop=(k == K - 1),
                        )
                        nc.tensor.matmul(
                            psb[:], w_sb[64:128, 128 * k : 128 * (k + 1)],
                            sg[64:128, c0 : c0 + 512],
                            start=(k == 0), stop=(k == K - 1),
                        )
                    o0 = 256 * it
                    nc.vector.tensor_reduce(
                        pla[:, o0 : o0 + 256],
                        psa[:].rearrange("p (n two) -> p n two", two=2),
                        mybir.AxisListType.X,
                        mybir.AluOpType.max,
                    )
                    nc.vector.tensor_reduce(
                        plb[:, o0 : o0 + 256],
                        psb[:].rearrange("p (n two) -> p n two", two=2),
                        mybir.AxisListType.X,
                        mybir.AluOpType.max,
                    )
                    # pooled-span flush: prelu (commutes with max) on ScalarE
                    # (coarse signs leave it ~14us of slack). Out DMAs on
                    # Sync; the scheduler hoists next-pair input DMAs past
                    # them. Last pair: finer pieces, final prelus split
                    # Scalar/Vector to shorten the tail.
                    if not last_pair:
                        spans = []
                        if it == 3:
                            spans = [(0, 1024)]
                        elif it == 7:
                            spans = [(1024, 1024)]
                        for s0, sw in spans:
                            for pl, b in ((pla, 2 * t), (plb, 2 * t + 1)):
                                o = opool.tile([128, sw], mybir.dt.bfloat16, name="o", tag="o")
                                nc.scalar.activation(
                                    o[:], pl[:, s0 : s0 + sw], AF.Prelu, alpha=slope
                                )
                                nc.sync.dma_start(
                                    oflat[128 * b : 128 * (b + 1), s0 : s0 + sw], o[:]
                                )
                    else:
                        spans = []
                        if it == 3:
                            spans = [(0, 1024)]
                        elif it == 5:
                            spans = [(1024, 512)]
                        elif it == 6:
                            spans = [(1536, 256)]
                        elif it == 7:
                            spans = [(1792, 256)]
                        for s0, sw in spans:
                            # sample A: prelu on Scalar
                            oa = opool.tile([128, sw], mybir.dt.bfloat16, name="oa", tag="o")
                            nc.scalar.activation(
                                oa[:], pla[:, s0 : s0 + sw], AF.Prelu, alpha=slope
                            )
                            nc.sync.dma_start(
                                oflat[128 * 2 * t : 128 * (2 * t + 1), s0 : s0 + sw],
                                oa[:],
                            )
                            # sample B: prelu on Vector (parallel with A;
                            # DVE reduces are done by the time these run)
                            ob = opool.tile([128, sw], mybir.dt.bfloat16, name="ob", tag="o")
                            nc.vector.scalar_tensor_tensor(
                                ob[:], plb[:, s0 : s0 + sw], slope,
                                plb[:, s0 : s0 + sw],
                                mybir.AluOpType.mult, mybir.AluOpType.max,
                            )
                            nc.sync.dma_start(
                                oflat[128 * (2 * t + 1) : 128 * (2 * t + 2), s0 : s0 + sw],
                                ob[:],
                            )
    return nc


def _split_sync_waits_json(bir: bytes) -> bytes:
    """Walrus in this toolchain accepts at most one sync-wait per instruction.
    Hoist multi-wait sync_info lists into preceding single-wait EventSemaphore
    instructions on the same engine queue (the same form engine.wait_ge()
    lowers to), preserving program order and on_update placement."""
    j = json.loads(bir)
    n_split = 0
    for fn in j.get("functions", []):
        for blk in fn.get("blocks", []):
            ins_list = blk.get("instructions")
            if not ins_list:
                continue
            out = []
            for ins in ins_list:
                si = ins.get("sync_info")
                waits = si.get("on_wait") if si else None
                if waits and len(waits) > 1:
                    for i, w in enumerate(waits):
                        out.append(
                            {
                                "debug": ins.get("debug", 0),
                                "engine": ins["engine"],
                                "ins": [],
                                "outs": [],
                                "name": f"{ins['name']}-antw{i}",
                                "opcode": "EventSemaphore",
                                "sync_info": {"on_update": [], "on_wait": [w]},
                            }
                        )
                    si["on_wait"] = []
                    n_split += 1
                out.append(ins)
            blk["instructions"] = out
    return json.dumps(j).encode()


def get_program() -> "bass.Bass":
    if "nc" not in _CACHE:
        nc = build_program()
        orig = nc.to_json_bytes
        nc.to_json_bytes = lambda: _split_sync_waits_json(orig())
        _CACHE["nc"] = nc
    return _CACHE["nc"]


def prep_inputs(I, bn_gamma, bn_beta, bn_mean, bn_var, conv_w, alpha, prelu_w):
    """Host-side folding: BN -> (scale, bias); alpha -> weights; per-k lhsT
    blocks duplicated into both PE array halves."""
    f32 = np.float32
    gamma = np.asarray(bn_gamma, f32)
    beta = np.asarray(bn_beta, f32)
    mean = np.asarray(bn_mean, f32)
    var = np.asarray(bn_var, f32)
    s = gamma / np.sqrt(var + f32(BN_EPS))        # [CIN]
    t = beta - mean * s                            # [CIN]

    w = np.asarray(conv_w, f32) * np.asarray(alpha, f32)[:, None, None]  # [COUT, CIN, K]
    Wb = np.zeros((128, K * 128), np.float32)
    for k in range(K):
        Wb[0:64, 128 * k : 128 * k + 128] = w[:, :, k].T
        Wb[64:128, 128 * k : 128 * k + 128] = w[:, :, k].T
    Wb = Wb.astype(ml_dtypes.bfloat16)

    a = f32(np.asarray(prelu_w, f32).reshape(-1)[0])
    sbp = np.zeros((128, 4), f32)
    sbp[0:64, 0] = s
    sbp[64:128, 0] = s
    sbp[0:64, 1] = t
    sbp[64:128, 1] = t
    sbp[:, 2] = f32(1.0) - a
    sbp[:, 3] = a
    return Wb, sbp


def kernel(I, bn_gamma, bn_beta, bn_mean, bn_var, conv_w, alpha, prelu_w):
    I = np.ascontiguousarray(np.asarray(I, np.float32))
    assert I.shape == (B, CIN, L), I.shape
    Wb, sbp = prep_inputs(I, bn_gamma, bn_beta, bn_mean, bn_var, conv_w, alpha, prelu_w)

    nc = get_program()
    in_maps = [
        {"I8": I[BPC * c : BPC * (c + 1)], "W": Wb, "SBp": sbp} for c in range(N_CORES)
    ]
    res = run_bass_kernel_spmd(nc, in_maps, core_ids=list(range(N_CORES)))
    out = np.concatenate(
        [np.asarray(res.results[c]["O8"]) for c in range(N_CORES)], axis=0
    )
    return np.ascontiguousarray(out.astype(np.float32))
